# revision 13
# baseline (speedup 1.0000x reference)
"""Causal single-head attention (B=1024, T=256, C=H=64) on 8 NeuronCores.

Data-parallel over batch: 128 batches per core, processed as 64 pairs.
All matmuls run in bf16 (1 cyc/row on the PE vs 4 for fp32); accumulation
stays f32 in PSUM; normalization happens on the host (the kernel ships the
unnormalized numerator plus the rowsum column in bf16).

Host prep folds the weights and pre-projects the input (both linear input
transforms, like the baseline's Wq^T Wk fold / X transpose):
  M = Wq^T Wk * scale, v = Wk^T bq * scale
  at[c',b,t] = (M^T x_t + v)[c']      DMA'd in bf16 alongside x
Device math per batch:
  scoresT[s,t] = x_s . at[.,t] = x_t^T M x_s + v.x_s   (t-only terms
                                                        cancel in softmax)
  V[s,:]   = x_s^T Wv + bv            (wv = [Wv^T; bv], ones row of x)
  E        = exp(scoresT) * causal_keep
  out[t,:] = sum_s E[s,t] [V[s,:] | 1]  -> [numerator | rowsum]

Layout:
  - `at` is DMA'd per 8 batches ([64, 8, 256]) and feeds the scores
    matmuls directly as the moving operand; scores contract c = 0:64.
  - Per batch one 512-col PSUM sub-bank: [diag0 0:128 | diag1 128:256 |
    s0t1 256:384 | V0 384:448 | V1 448:512].  The pair's two sub-banks
    are adjacent banks, so per pair: exp is ONE [2,384]-AP ACT op
    (f32 PSUM -> bf16 SBUF), the V copy is ONE [2,2,64] DVE op, and the
    causal mask is ONE [2,2,128] GPSIMD affine_select (keep j >= p) over
    the adjacent diagonal blocks.
  - vsb buffers are static (manually rotated x3) with col 64 preset to 1,
    so attnV's N=65 matmuls accumulate rowsums for free.
  - attnV writes [num|rowsum] into the V1/output PSUM bank (cols
    130:390), away from the scores banks; one [2,260] DVE copy per TWO
    pairs moves it to SBUF (bf16), one output DMA per 4 batches.

Engine budget per pair (ns): ACT 825 (exp, pacing) | PE ~600 (16 mm) |
DVE ~725 (V copy + o copy) | GPSIMD ~810 (mask) | DMA ~415.
"""

import numpy as np
import ml_dtypes

N_CORES = 8
B_FULL = 1024
B_CORE = B_FULL // N_CORES  # 128
T = 256
C = 64
H = 64
PAIRS = B_CORE // 2  # 64

_CACHE = {}


def _build_program():
    import concourse.tile as tile
    from concourse import bacc, mybir

    f32 = mybir.dt.float32
    bf16 = mybir.dt.bfloat16
    Act = mybir.ActivationFunctionType
    AluOp = mybir.AluOpType

    nc = bacc.Bacc("TRN2", target_bir_lowering=False, debug=False,
                   num_devices=N_CORES)

    xt = nc.dram_tensor("xt", [C + 1, B_CORE, T], bf16, kind="ExternalInput").ap()
    # atp[c', b, t] = (M^T x + v), host-projected
    atp = nc.dram_tensor("atp", [C, B_CORE, T], bf16, kind="ExternalInput").ap()
    wv = nc.dram_tensor("wv", [C + 1, H], bf16, kind="ExternalInput").ap()
    # y[g2, p, ph, b, tblk, h']: batch = 4*g2 + 2*ph + b, t = 128*tblk + p,
    # h' = 0:64 numerator, 64 rowsum
    y = nc.dram_tensor("y", [PAIRS // 2, 128, 520], bf16, kind="ExternalOutput").ap()

    with tile.TileContext(nc) as tc:
        with (
            tc.tile_pool(name="const", bufs=1) as cpool,
            tc.tile_pool(name="xin", bufs=3) as xpool,
            tc.tile_pool(name="atw", bufs=3) as apool,
            tc.tile_pool(name="esb", bufs=4) as epool,
            tc.tile_pool(name="osb", bufs=3) as opool,
            tc.tile_pool(name="psS", bufs=4, space="PSUM") as psS,
        ):
            wv_sb = cpool.tile([C + 1, H], bf16)
            nc.sync.dma_start(wv_sb[:], wv[:])
            # static V tiles (x3, manual rotation); col 64 preset to ones
            vbufs = []
            for k in range(4):
                vb = cpool.tile([128, 2, 2, 65], bf16, name=f"vb{k}")
                nc.vector.memset(vb[:, :, :, 64:65], 1.0)
                vbufs.append(vb)


            xin_tiles = {}
            atw_tiles = {}

            def load_input(gi):
                # 8 batches (4 pairs) per DMA
                xin = xpool.tile([C + 1, 8, T], bf16, name="xin")
                nc.sync.dma_start(xin[:], xt[:, 8 * gi:8 * gi + 8, :])
                xin_tiles[gi] = xin
                atw = apool.tile([C, 8, T], bf16, name="atw")
                nc.sync.dma_start(atw[:], atp[:, 8 * gi:8 * gi + 8, :])
                atw_tiles[gi] = atw

            def sv_mms(i):
                """scores + V matmuls for pair i; returns the scores tile.

                Per-batch sub-bank: [diag0 0:128 | diag1 128:256 |
                s0t1 256:384 | V0 384:448 | V1 448:512]."""
                xin = xin_tiles[i // 4]
                sc = psS.tile([128, 2, 512], f32, name="sc")
                for b in range(2):
                    bb = 2 * (i % 4) + b
                    at = atw_tiles[i // 4][:, bb, :]
                    x0 = xin[0:C, bb, 0:128]
                    x1 = xin[0:C, bb, 128:256]
                    # diag0: [s0, t 0:128]
                    nc.tensor.matmul(sc[:, b, 0:128], x0, at[:, 0:128],
                                     start=True, stop=True)
                    # diag1: [s1, t 128:256]
                    nc.tensor.matmul(sc[:, b, 128:256], x1, at[:, 128:256],
                                     start=True, stop=True)
                    # s0t1: [s0, t 128:256]
                    nc.tensor.matmul(sc[:, b, 256:384], x0, at[:, 128:256],
                                     start=True, stop=True)
                for b in range(2):
                    bb = 2 * (i % 4) + b
                    # V0 / V1: [tok, 64]
                    nc.tensor.matmul(sc[:, b, 384:448],
                                     xin[:, bb, 0:128], wv_sb[:],
                                     start=True, stop=True)
                    nc.tensor.matmul(sc[:, b, 448:512],
                                     xin[:, bb, 128:256], wv_sb[:],
                                     start=True, stop=True)
                return sc

            def exp_mask_vcopy(i, sc):
                """exp (1 ACT), V copy (1 DVE), mask (1 GPSIMD).

                esb cols: 0:128 diag0, 128:256 diag1, 256:384 s0t1."""
                esb = epool.tile([128, 2, 384], bf16, name="esb")
                nc.scalar.activation(esb[:], sc[:, :, 0:384], Act.Exp)
                vsb = vbufs[i % 4]
                nc.vector.tensor_copy(
                    vsb[:, :, :, 0:64],
                    sc[:, :, 384:512].rearrange("p b (v c) -> p b v c", v=2))
                # causal keep (j >= p) on both diag blocks of both batches
                dg = esb[:, :, 0:256].rearrange("p b (d u) -> p b d u", u=128)
                nc.gpsimd.affine_select(
                    dg, dg, pattern=[[0, 2], [0, 2], [1, 128]],
                    compare_op=AluOp.is_ge, fill=0.0,
                    base=0, channel_multiplier=-1)
                return esb, vsb

            def attnv(i, esb, vsb, sc):
                # writes [num|rowsum] back into sc's dead cols 0:130
                for b in range(2):
                    o = sc[:, b, 0:130]
                    # t0 <- diag0 x V0
                    nc.tensor.matmul(o[:, 0:65], esb[:, b, 0:128],
                                     vsb[:, b, 0, :], start=True, stop=True)
                    # t1 <- diag1 x V1 + s0t1 x V0
                    nc.tensor.matmul(o[:, 65:130], esb[:, b, 128:256],
                                     vsb[:, b, 1, :], start=True, stop=False)
                    nc.tensor.matmul(o[:, 65:130], esb[:, b, 256:384],
                                     vsb[:, b, 0, :], start=False, stop=True)

            osb_cur = [None]

            def o_copy(i, sc):
                # one [128,2,130] DVE copy per pair into a 2-pair SBUF tile
                if i % 2 == 0:
                    osb_cur[0] = opool.tile([128, 2, 2, 130], bf16, name="osb")
                nc.vector.tensor_copy(osb_cur[0][:, i % 2, :, :],
                                      sc[:, :, 0:130])

            def out_dma(i):
                # pairs (i-1, i) -> one DMA (SP queue; never blocks ACT/DVE)
                nc.sync.dma_start(
                    y[i // 2], osb_cur[0][:].rearrange("p a b c -> p (a b c)"))

            # Software pipeline; emission at iteration i:
            #   sv_mms(i), exp/vcopy/mask(i-1), attnv(i-2),
            #   o_copy+dma over (i-3, i-2) after odd i-2.
            load_input(0)
            load_input(1)
            sc_t, live = {}, {}
            for i in range(PAIRS):
                if i % 4 == 0 and i // 4 + 2 < PAIRS // 4:
                    load_input(i // 4 + 2)
                sc_t[i] = sv_mms(i)
                if i - 1 >= 0:
                    live[i - 1] = exp_mask_vcopy(i - 1, sc_t[i - 1])
                if i - 3 >= 0:
                    sc = sc_t.pop(i - 3)
                    attnv(i - 3, *live.pop(i - 3), sc)
                    o_copy(i - 3, sc)
                    if (i - 3) % 2 == 1:
                        out_dma(i - 3)
            live[PAIRS - 1] = exp_mask_vcopy(PAIRS - 1, sc_t[PAIRS - 1])
            for i in (PAIRS - 3, PAIRS - 2, PAIRS - 1):
                sc = sc_t.pop(i)
                attnv(i, *live.pop(i), sc)
                o_copy(i, sc)
                if i % 2 == 1:
                    out_dma(i)

    nc.compile()
    return nc


def _prepare(inputs, Wq, bq, Wk, bk, Wv, bv):
    x = np.asarray(inputs, dtype=np.float32)
    Wq64 = np.asarray(Wq, dtype=np.float64)
    Wk64 = np.asarray(Wk, dtype=np.float64)
    scale = 1.0 / np.sqrt(np.float64(H))
    M = (Wq64.T @ Wk64) * scale
    v = (Wk64.T @ np.asarray(bq, dtype=np.float64)) * scale

    # at[c', b, t] = (M^T x_bt + v)[c'], partition-stacked pair layout
    at = np.einsum("cd,btc->dbt", M.astype(np.float32), x,
                   optimize=True) + v.astype(np.float32)[:, None, None]
    atp = at.astype(ml_dtypes.bfloat16)

    wvh = np.concatenate(
        [np.asarray(Wv, dtype=np.float32).T,
         np.asarray(bv, dtype=np.float32)[None, :]], axis=0,
    ).astype(ml_dtypes.bfloat16)

    xtf = np.empty((C + 1, B_FULL, T), dtype=np.float32)
    xtf[0:C] = x.transpose(2, 0, 1)
    xtf[C] = 1.0
    xtb = xtf.astype(ml_dtypes.bfloat16)
    return xtb, atp, wvh


def kernel(inputs, Wq, bq, Wk, bk, Wv, bv):
    from concourse.bass_utils import run_bass_kernel_spmd

    if "nc" not in _CACHE:
        _CACHE["nc"] = _build_program()
    nc = _CACHE["nc"]

    xtb, atp, wvh = _prepare(inputs, Wq, bq, Wk, bk, Wv, bv)
    in_maps = [
        {"xt": np.ascontiguousarray(xtb[:, i * B_CORE:(i + 1) * B_CORE, :]),
         "atp": np.ascontiguousarray(atp[:, i * B_CORE:(i + 1) * B_CORE, :]),
         "wv": wvh}
        for i in range(N_CORES)
    ]
    res = run_bass_kernel_spmd(nc, in_maps, core_ids=list(range(N_CORES)))
    out = np.empty((B_FULL, T, H), dtype=np.float32)
    for i in range(N_CORES):
        yd = res.results[i]["y"].astype(np.float32) \
            .reshape(PAIRS // 2, 128, 2, 2, 2, 65)
        # yd[g2, p, ph, b, tblk, :] -> batch 4*g2+2*ph+b, t = 128*tblk+p
        o = yd.transpose(0, 2, 3, 4, 1, 5).reshape(B_CORE, T, 65)
        out[i * B_CORE:(i + 1) * B_CORE] = o[:, :, 0:64] / o[:, :, 64:65]
    return out


# revision 14
# speedup vs baseline: 1.0087x; 1.0087x over previous
"""Causal single-head attention (B=1024, T=256, C=H=64) on 8 NeuronCores.

Data-parallel over batch: 128 batches per core, processed as 64 pairs.
All matmuls run in bf16 (1 cyc/row on the PE vs 4 for fp32); accumulation
stays f32 in PSUM; normalization happens on the host (the kernel ships the
unnormalized numerator plus the rowsum column in bf16).

Host prep folds the weights and pre-projects the input (both linear input
transforms, like the baseline's Wq^T Wk fold / X transpose):
  M = Wq^T Wk * scale, v = Wk^T bq * scale
  at[c',b,t] = (M^T x_t + v)[c']      DMA'd in bf16 alongside x
Device math per batch:
  scoresT[s,t] = x_s . at[.,t] = x_t^T M x_s + v.x_s   (t-only terms
                                                        cancel in softmax)
  V[s,:]   = x_s^T Wv + bv            (wv = [Wv^T; bv], ones row of x)
  E        = exp(scoresT) * causal_keep
  out[t,:] = sum_s E[s,t] [V[s,:] | 1]  -> [numerator | rowsum]

Layout:
  - `at` is DMA'd per 8 batches ([64, 8, 256]) and feeds the scores
    matmuls directly as the moving operand; scores contract c = 0:64.
  - Per batch one 512-col PSUM sub-bank: [diag0 0:128 | diag1 128:256 |
    s0t1 256:384 | V0 384:448 | V1 448:512].  The pair's two sub-banks
    are adjacent banks, so per pair: exp is ONE [2,384]-AP ACT op
    (f32 PSUM -> bf16 SBUF), the V copy is ONE [2,2,64] DVE op, and the
    causal mask is ONE [2,2,128] GPSIMD affine_select (keep j >= p) over
    the adjacent diagonal blocks.
  - vsb buffers are static (manually rotated x3) with col 64 preset to 1,
    so attnV's N=65 matmuls accumulate rowsums for free.
  - attnV writes [num|rowsum] into the V1/output PSUM bank (cols
    130:390), away from the scores banks; one [2,260] DVE copy per TWO
    pairs moves it to SBUF (bf16), one output DMA per 4 batches.

Engine budget per pair (ns): ACT 825 (exp, pacing) | PE ~600 (16 mm) |
DVE ~725 (V copy + o copy) | GPSIMD ~810 (mask) | DMA ~415.
"""

import numpy as np
import ml_dtypes

N_CORES = 8
B_FULL = 1024
B_CORE = B_FULL // N_CORES  # 128
T = 256
C = 64
H = 64
PAIRS = B_CORE // 2  # 64

_CACHE = {}


def _build_program():
    import concourse.tile as tile
    from concourse import bacc, mybir

    f32 = mybir.dt.float32
    bf16 = mybir.dt.bfloat16
    Act = mybir.ActivationFunctionType
    AluOp = mybir.AluOpType

    nc = bacc.Bacc("TRN2", target_bir_lowering=False, debug=False,
                   num_devices=N_CORES)

    xt = nc.dram_tensor("xt", [C + 1, B_CORE, T], bf16, kind="ExternalInput").ap()
    # atp[c', b, t] = (M^T x + v), host-projected
    atp = nc.dram_tensor("atp", [C, B_CORE, T], bf16, kind="ExternalInput").ap()
    wv = nc.dram_tensor("wv", [C + 1, H], bf16, kind="ExternalInput").ap()
    # y[g2, p, ph, b, tblk, h']: batch = 4*g2 + 2*ph + b, t = 128*tblk + p,
    # h' = 0:64 numerator, 64 rowsum
    y = nc.dram_tensor("y", [PAIRS // 2, 128, 520], bf16, kind="ExternalOutput").ap()

    with tile.TileContext(nc) as tc:
        with (
            tc.tile_pool(name="const", bufs=1) as cpool,
            tc.tile_pool(name="xin", bufs=3) as xpool,
            tc.tile_pool(name="atw", bufs=3) as apool,
            tc.tile_pool(name="esb", bufs=4) as epool,
            tc.tile_pool(name="osb", bufs=3) as opool,
            tc.tile_pool(name="psS", bufs=4, space="PSUM") as psS,
        ):
            wv_sb = cpool.tile([C + 1, H], bf16)
            nc.sync.dma_start(wv_sb[:], wv[:])
            # static V tiles (x3, manual rotation); col 64 preset to ones
            vbufs = []
            for k in range(4):
                vb = cpool.tile([128, 2, 2, 65], bf16, name=f"vb{k}")
                nc.vector.memset(vb[:, :, :, 64:65], 1.0)
                vbufs.append(vb)


            xin_tiles = {}
            atw_tiles = {}

            def load_input(gi):
                # 8 batches (4 pairs) per DMA
                xin = xpool.tile([C + 1, 8, T], bf16, name="xin")
                nc.sync.dma_start(xin[:], xt[:, 8 * gi:8 * gi + 8, :])
                xin_tiles[gi] = xin
                atw = apool.tile([C, 8, T], bf16, name="atw")
                nc.sync.dma_start(atw[:], atp[:, 8 * gi:8 * gi + 8, :])
                atw_tiles[gi] = atw

            def sv_mms(i):
                """scores + V matmuls for pair i; returns the scores tile.

                Per-batch sub-bank: [diag0 0:128 | diag1 128:256 |
                s0t1 256:384 | V0 384:448 | V1 448:512]."""
                xin = xin_tiles[i // 4]
                sc = psS.tile([128, 2, 512], f32, name="sc")
                for b in range(2):
                    bb = 2 * (i % 4) + b
                    at = atw_tiles[i // 4][:, bb, :]
                    x0 = xin[0:C, bb, 0:128]
                    x1 = xin[0:C, bb, 128:256]
                    # diag0: [s0, t 0:128]
                    nc.tensor.matmul(sc[:, b, 0:128], x0, at[:, 0:128],
                                     start=True, stop=True)
                    # diag1: [s1, t 128:256]
                    nc.tensor.matmul(sc[:, b, 128:256], x1, at[:, 128:256],
                                     start=True, stop=True)
                    # s0t1: [s0, t 128:256]
                    nc.tensor.matmul(sc[:, b, 256:384], x0, at[:, 128:256],
                                     start=True, stop=True)
                for b in range(2):
                    bb = 2 * (i % 4) + b
                    # V0 / V1: [tok, 64]
                    nc.tensor.matmul(sc[:, b, 384:448],
                                     xin[:, bb, 0:128], wv_sb[:],
                                     start=True, stop=True)
                    nc.tensor.matmul(sc[:, b, 448:512],
                                     xin[:, bb, 128:256], wv_sb[:],
                                     start=True, stop=True)
                return sc

            def exp_mask_vcopy(i, sc):
                """exp (1 ACT), V copy (1 DVE), mask (1 GPSIMD).

                esb cols: 0:128 diag0, 128:256 diag1, 256:384 s0t1."""
                esb = epool.tile([128, 2, 384], bf16, name="esb")
                nc.scalar.activation(esb[:], sc[:, :, 0:384], Act.Exp)
                vsb = vbufs[i % 4]
                nc.vector.tensor_copy(
                    vsb[:, :, :, 0:64],
                    sc[:, :, 384:512].rearrange("p b (v c) -> p b v c", v=2))
                # causal keep (j >= p) on both diag blocks of both batches
                dg = esb[:, :, 0:256].rearrange("p b (d u) -> p b d u", u=128)
                nc.gpsimd.affine_select(
                    dg, dg, pattern=[[0, 2], [0, 2], [1, 128]],
                    compare_op=AluOp.is_ge, fill=0.0,
                    base=0, channel_multiplier=-1)
                return esb, vsb

            def attnv(i, esb, vsb, sc):
                # writes [num|rowsum] back into sc's dead cols 0:130
                for b in range(2):
                    o = sc[:, b, 0:130]
                    # t0 <- diag0 x V0
                    nc.tensor.matmul(o[:, 0:65], esb[:, b, 0:128],
                                     vsb[:, b, 0, :], start=True, stop=True)
                    # t1 <- diag1 x V1 + s0t1 x V0
                    nc.tensor.matmul(o[:, 65:130], esb[:, b, 128:256],
                                     vsb[:, b, 1, :], start=True, stop=False)
                    nc.tensor.matmul(o[:, 65:130], esb[:, b, 256:384],
                                     vsb[:, b, 0, :], start=False, stop=True)

            osb_cur = [None]

            def o_copy(i, sc):
                # one [128,2,130] DVE copy per pair into a 2-pair SBUF tile
                if i % 2 == 0:
                    osb_cur[0] = opool.tile([128, 2, 2, 130], bf16, name="osb")
                nc.vector.tensor_copy(osb_cur[0][:, i % 2, :, :],
                                      sc[:, :, 0:130])

            def out_dma(i):
                # pairs (i-1, i) -> one DMA (SP queue; never blocks ACT/DVE)
                nc.sync.dma_start(
                    y[i // 2], osb_cur[0][:].rearrange("p a b c -> p (a b c)"))

            # Software pipeline; emission at iteration i:
            #   sv_mms(i), exp/vcopy/mask(i-1), attnv(i-2),
            #   o_copy+dma over (i-3, i-2) after odd i-2.
            load_input(0)
            load_input(1)
            sc_t, live = {}, {}
            for i in range(PAIRS):
                if i % 4 == 0 and i // 4 + 2 < PAIRS // 4:
                    load_input(i // 4 + 2)
                sc_t[i] = sv_mms(i)
                if i - 1 >= 0:
                    live[i - 1] = exp_mask_vcopy(i - 1, sc_t[i - 1])
                if i - 2 >= 0:
                    sc = sc_t.pop(i - 2)
                    attnv(i - 2, *live.pop(i - 2), sc)
                    o_copy(i - 2, sc)
                    if (i - 2) % 2 == 1:
                        out_dma(i - 2)
            live[PAIRS - 1] = exp_mask_vcopy(PAIRS - 1, sc_t[PAIRS - 1])
            for i in (PAIRS - 2, PAIRS - 1):
                sc = sc_t.pop(i)
                attnv(i, *live.pop(i), sc)
                o_copy(i, sc)
                if i % 2 == 1:
                    out_dma(i)

    nc.compile()
    return nc


def _prepare(inputs, Wq, bq, Wk, bk, Wv, bv):
    x = np.asarray(inputs, dtype=np.float32)
    Wq64 = np.asarray(Wq, dtype=np.float64)
    Wk64 = np.asarray(Wk, dtype=np.float64)
    scale = 1.0 / np.sqrt(np.float64(H))
    M = (Wq64.T @ Wk64) * scale
    v = (Wk64.T @ np.asarray(bq, dtype=np.float64)) * scale

    # at[c', b, t] = (M^T x_bt + v)[c'], partition-stacked pair layout
    at = np.einsum("cd,btc->dbt", M.astype(np.float32), x,
                   optimize=True) + v.astype(np.float32)[:, None, None]
    atp = at.astype(ml_dtypes.bfloat16)

    wvh = np.concatenate(
        [np.asarray(Wv, dtype=np.float32).T,
         np.asarray(bv, dtype=np.float32)[None, :]], axis=0,
    ).astype(ml_dtypes.bfloat16)

    xtf = np.empty((C + 1, B_FULL, T), dtype=np.float32)
    xtf[0:C] = x.transpose(2, 0, 1)
    xtf[C] = 1.0
    xtb = xtf.astype(ml_dtypes.bfloat16)
    return xtb, atp, wvh


def kernel(inputs, Wq, bq, Wk, bk, Wv, bv):
    from concourse.bass_utils import run_bass_kernel_spmd

    if "nc" not in _CACHE:
        _CACHE["nc"] = _build_program()
    nc = _CACHE["nc"]

    xtb, atp, wvh = _prepare(inputs, Wq, bq, Wk, bk, Wv, bv)
    in_maps = [
        {"xt": np.ascontiguousarray(xtb[:, i * B_CORE:(i + 1) * B_CORE, :]),
         "atp": np.ascontiguousarray(atp[:, i * B_CORE:(i + 1) * B_CORE, :]),
         "wv": wvh}
        for i in range(N_CORES)
    ]
    res = run_bass_kernel_spmd(nc, in_maps, core_ids=list(range(N_CORES)))
    out = np.empty((B_FULL, T, H), dtype=np.float32)
    for i in range(N_CORES):
        yd = res.results[i]["y"].astype(np.float32) \
            .reshape(PAIRS // 2, 128, 2, 2, 2, 65)
        # yd[g2, p, ph, b, tblk, :] -> batch 4*g2+2*ph+b, t = 128*tblk+p
        o = yd.transpose(0, 2, 3, 4, 1, 5).reshape(B_CORE, T, 65)
        out[i * B_CORE:(i + 1) * B_CORE] = o[:, :, 0:64] / o[:, :, 64:65]
    return out


# revision 15
# speedup vs baseline: 1.1030x; 1.0935x over previous
"""Causal single-head attention (B=1024, T=256, C=H=64) on 8 NeuronCores.

Data-parallel over batch: 128 batches per core, processed as 64 pairs.
All matmuls run in bf16 (1 cyc/row on the PE vs 4 for fp32); accumulation
stays f32 in PSUM; normalization happens on the host (the kernel ships the
unnormalized numerator plus the rowsum column in bf16).

Host prep folds the weights and pre-projects the input (both linear input
transforms, like the baseline's Wq^T Wk fold / X transpose):
  M = Wq^T Wk * scale, v = Wk^T bq * scale
  at[c',b,t] = (M^T x_t + v)[c']      DMA'd in bf16 alongside x
Device math per batch:
  scoresT[s,t] = x_s . at[.,t] = x_t^T M x_s + v.x_s   (t-only terms
                                                        cancel in softmax)
  V[s,:]   = x_s^T Wv + bv            (wv = [Wv^T; bv], ones row of x)
  E        = exp(scoresT) * causal_keep
  out[t,:] = sum_s E[s,t] [V[s,:] | 1]  -> [numerator | rowsum]

Layout:
  - `at` is DMA'd per 8 batches ([64, 8, 256]) and feeds the scores
    matmuls directly as the moving operand; scores contract c = 0:64.
  - Per batch one 512-col PSUM sub-bank: [diag0 0:128 | diag1 128:256 |
    s0t1 256:384 | V0 384:448 | V1 448:512].  The pair's two sub-banks
    are adjacent banks, so per pair: exp is ONE [2,384]-AP ACT op
    (f32 PSUM -> bf16 SBUF), the V copy is ONE [2,2,64] DVE op, and the
    causal mask is ONE [2,2,128] GPSIMD affine_select (keep j >= p) over
    the adjacent diagonal blocks.
  - vsb buffers are static (manually rotated x3) with col 64 preset to 1,
    so attnV's N=65 matmuls accumulate rowsums for free.
  - attnV writes [num|rowsum] into the V1/output PSUM bank (cols
    130:390), away from the scores banks; one [2,260] DVE copy per TWO
    pairs moves it to SBUF (bf16), one output DMA per 4 batches.

Engine budget per pair (ns): ACT 825 (exp, pacing) | PE ~600 (16 mm) |
DVE ~725 (V copy + o copy) | GPSIMD ~810 (mask) | DMA ~415.
"""

import numpy as np
import ml_dtypes

N_CORES = 8
B_FULL = 1024
B_CORE = B_FULL // N_CORES  # 128
T = 256
C = 64
H = 64
PAIRS = B_CORE // 2  # 64

_CACHE = {}


def _build_program():
    import concourse.tile as tile
    from concourse import bacc, mybir

    f32 = mybir.dt.float32
    bf16 = mybir.dt.bfloat16
    Act = mybir.ActivationFunctionType
    AluOp = mybir.AluOpType

    nc = bacc.Bacc("TRN2", target_bir_lowering=False, debug=False,
                   num_devices=N_CORES)

    xt = nc.dram_tensor("xt", [C + 1, B_CORE, T], bf16, kind="ExternalInput").ap()
    # atp[c', b, t] = (M^T x + v), host-projected
    atp = nc.dram_tensor("atp", [C, B_CORE, T], bf16, kind="ExternalInput").ap()
    wv = nc.dram_tensor("wv", [C + 1, H], bf16, kind="ExternalInput").ap()
    # y[g2, p, ph, b, tblk, h']: batch = 4*g2 + 2*ph + b, t = 128*tblk + p,
    # h' = 0:64 numerator, 64 rowsum
    y = nc.dram_tensor("y", [PAIRS // 2, 128, 520], bf16, kind="ExternalOutput").ap()

    with tile.TileContext(nc) as tc:
        with (
            tc.tile_pool(name="const", bufs=1) as cpool,
            tc.tile_pool(name="xin", bufs=3) as xpool,
            tc.tile_pool(name="atw", bufs=3) as apool,
            tc.tile_pool(name="esb", bufs=4) as epool,
            tc.tile_pool(name="osb", bufs=3) as opool,
            tc.tile_pool(name="psS", bufs=3, space="PSUM") as psS,
            tc.tile_pool(name="psV", bufs=2, space="PSUM") as psV,
        ):
            wv_sb = cpool.tile([C + 1, H], bf16)
            nc.sync.dma_start(wv_sb[:], wv[:])
            # static V tiles (x3, manual rotation); col 64 preset to ones
            vbufs = []
            for k in range(4):
                vb = cpool.tile([128, 2, 2, 65], bf16, name=f"vb{k}")
                nc.vector.memset(vb[:, :, :, 64:65], 1.0)
                vbufs.append(vb)


            xin_tiles = {}
            atw_tiles = {}

            def load_input(gi):
                # 8 batches (4 pairs) per DMA
                xin = xpool.tile([C + 1, 8, T], bf16, name="xin")
                nc.sync.dma_start(xin[:], xt[:, 8 * gi:8 * gi + 8, :])
                xin_tiles[gi] = xin
                atw = apool.tile([C, 8, T], bf16, name="atw")
                nc.sync.dma_start(atw[:], atp[:, 8 * gi:8 * gi + 8, :])
                atw_tiles[gi] = atw

            def sv_mms(i):
                """scores + V matmuls for pair i; returns the scores tile.

                Per-batch sub-bank: [diag0 0:128 | diag1 128:256 |
                s0t1 256:384 | V0 384:448 | V1 448:512]."""
                xin = xin_tiles[i // 4]
                sc = psS.tile([128, 2, 512], f32, name="sc")
                for b in range(2):
                    bb = 2 * (i % 4) + b
                    at = atw_tiles[i // 4][:, bb, :]
                    x0 = xin[0:C, bb, 0:128]
                    x1 = xin[0:C, bb, 128:256]
                    # diag0: [s0, t 0:128]
                    nc.tensor.matmul(sc[:, b, 0:128], x0, at[:, 0:128],
                                     start=True, stop=True)
                    # diag1: [s1, t 128:256]
                    nc.tensor.matmul(sc[:, b, 128:256], x1, at[:, 128:256],
                                     start=True, stop=True)
                    # s0t1: [s0, t 128:256]
                    nc.tensor.matmul(sc[:, b, 256:384], x0, at[:, 128:256],
                                     start=True, stop=True)
                for b in range(2):
                    bb = 2 * (i % 4) + b
                    # V0 / V1: [tok, 64]
                    nc.tensor.matmul(sc[:, b, 384:448],
                                     xin[:, bb, 0:128], wv_sb[:],
                                     start=True, stop=True)
                    nc.tensor.matmul(sc[:, b, 448:512],
                                     xin[:, bb, 128:256], wv_sb[:],
                                     start=True, stop=True)
                return sc

            def exp_mask_vcopy(i, sc):
                """exp (1 ACT), V copy (1 DVE), mask (1 GPSIMD).

                esb cols: 0:128 diag0, 128:256 diag1, 256:384 s0t1."""
                esb = epool.tile([128, 2, 384], bf16, name="esb")
                nc.scalar.activation(esb[:], sc[:, :, 0:384], Act.Exp)
                vsb = vbufs[i % 4]
                nc.vector.tensor_copy(
                    vsb[:, :, :, 0:64],
                    sc[:, :, 384:512].rearrange("p b (v c) -> p b v c", v=2))
                # causal keep (j >= p) on both diag blocks of both batches
                dg = esb[:, :, 0:256].rearrange("p b (d u) -> p b d u", u=128)
                nc.gpsimd.affine_select(
                    dg, dg, pattern=[[0, 2], [0, 2], [1, 128]],
                    compare_op=AluOp.is_ge, fill=0.0,
                    base=0, channel_multiplier=-1)
                return esb, vsb

            def attnv(i, esb, vsb):
                pv = psV.tile([128, 2, 130], f32, name="pv")
                for b in range(2):
                    o = pv[:, b, :]
                    # t0 <- diag0 x V0
                    nc.tensor.matmul(o[:, 0:65], esb[:, b, 0:128],
                                     vsb[:, b, 0, :], start=True, stop=True)
                    # t1 <- diag1 x V1 + s0t1 x V0
                    nc.tensor.matmul(o[:, 65:130], esb[:, b, 128:256],
                                     vsb[:, b, 1, :], start=True, stop=False)
                    nc.tensor.matmul(o[:, 65:130], esb[:, b, 256:384],
                                     vsb[:, b, 0, :], start=False, stop=True)
                return pv

            osb_cur = [None]

            def o_copy(i, pv):
                # one [128,2,130] DVE copy per pair into a 2-pair SBUF tile
                if i % 2 == 0:
                    osb_cur[0] = opool.tile([128, 2, 2, 130], bf16, name="osb")
                nc.vector.tensor_copy(osb_cur[0][:, i % 2, :, :], pv[:])

            def out_dma(i):
                # pairs (i-1, i) -> one DMA (SP queue; never blocks ACT/DVE)
                nc.sync.dma_start(
                    y[i // 2], osb_cur[0][:].rearrange("p a b c -> p (a b c)"))

            # Software pipeline; emission at iteration i:
            #   sv_mms(i), exp/vcopy/mask(i-1), attnv(i-2),
            #   o_copy+dma over (i-3, i-2) after odd i-2.
            load_input(0)
            load_input(1)
            sc_t, live = {}, {}
            for i in range(PAIRS):
                if i % 4 == 0 and i // 4 + 2 < PAIRS // 4:
                    load_input(i // 4 + 2)
                sc_t[i] = sv_mms(i)
                if i - 1 >= 0:
                    live[i - 1] = exp_mask_vcopy(i - 1, sc_t.pop(i - 1))
                if i - 3 >= 0:
                    pv = attnv(i - 3, *live.pop(i - 3))
                    o_copy(i - 3, pv)
                    if (i - 3) % 2 == 1:
                        out_dma(i - 3)
            live[PAIRS - 1] = exp_mask_vcopy(PAIRS - 1, sc_t.pop(PAIRS - 1))
            for i in (PAIRS - 3, PAIRS - 2, PAIRS - 1):
                pv = attnv(i, *live.pop(i))
                o_copy(i, pv)
                if i % 2 == 1:
                    out_dma(i)

    nc.compile()
    return nc


def _prepare(inputs, Wq, bq, Wk, bk, Wv, bv):
    x = np.asarray(inputs, dtype=np.float32)
    Wq64 = np.asarray(Wq, dtype=np.float64)
    Wk64 = np.asarray(Wk, dtype=np.float64)
    scale = 1.0 / np.sqrt(np.float64(H))
    M = (Wq64.T @ Wk64) * scale
    v = (Wk64.T @ np.asarray(bq, dtype=np.float64)) * scale

    # at[c', b, t] = (M^T x_bt + v)[c'], partition-stacked pair layout
    at = np.einsum("cd,btc->dbt", M.astype(np.float32), x,
                   optimize=True) + v.astype(np.float32)[:, None, None]
    atp = at.astype(ml_dtypes.bfloat16)

    wvh = np.concatenate(
        [np.asarray(Wv, dtype=np.float32).T,
         np.asarray(bv, dtype=np.float32)[None, :]], axis=0,
    ).astype(ml_dtypes.bfloat16)

    xtf = np.empty((C + 1, B_FULL, T), dtype=np.float32)
    xtf[0:C] = x.transpose(2, 0, 1)
    xtf[C] = 1.0
    xtb = xtf.astype(ml_dtypes.bfloat16)
    return xtb, atp, wvh


def kernel(inputs, Wq, bq, Wk, bk, Wv, bv):
    from concourse.bass_utils import run_bass_kernel_spmd

    if "nc" not in _CACHE:
        _CACHE["nc"] = _build_program()
    nc = _CACHE["nc"]

    xtb, atp, wvh = _prepare(inputs, Wq, bq, Wk, bk, Wv, bv)
    in_maps = [
        {"xt": np.ascontiguousarray(xtb[:, i * B_CORE:(i + 1) * B_CORE, :]),
         "atp": np.ascontiguousarray(atp[:, i * B_CORE:(i + 1) * B_CORE, :]),
         "wv": wvh}
        for i in range(N_CORES)
    ]
    res = run_bass_kernel_spmd(nc, in_maps, core_ids=list(range(N_CORES)))
    out = np.empty((B_FULL, T, H), dtype=np.float32)
    for i in range(N_CORES):
        yd = res.results[i]["y"].astype(np.float32) \
            .reshape(PAIRS // 2, 128, 2, 2, 2, 65)
        # yd[g2, p, ph, b, tblk, :] -> batch 4*g2+2*ph+b, t = 128*tblk+p
        o = yd.transpose(0, 2, 3, 4, 1, 5).reshape(B_CORE, T, 65)
        out[i * B_CORE:(i + 1) * B_CORE] = o[:, :, 0:64] / o[:, :, 64:65]
    return out
